# revision 3
# baseline (speedup 1.0000x reference)
"""GATv2Conv (heads=1, edge_dim=11, add_self_loops fill='mean') on 8 Trainium2 cores.

Sharding: edges partitioned by destination-node ownership (12500 nodes/core), so
every segment reduction (softmax denominator, degree, attr-sum, weighted output
sum) is core-local. Per 128-node destination chunk, segment reductions are
one-hot matmuls (S^T @ rhs) accumulated in PSUM -- no scatter DMA, no
collectives, no segment-max (randn-scale logits make exp safe).

Per-edge xl[src] / xr[dst] rows are fetched with the dma_gather SWDGE ucode
from on-device bf16 tables C[n]=[xl|xr] and XR[j]=[xr|xl] (own nodes).
src indices are split over 4 row-banks of 25024 to fit int16.

Host work is layout only: sharding, grouping, index packing, transposes.
"""

import sys

sys.path.insert(0, "/opt/trn_rl_repo")

import numpy as np
import ml_dtypes

import concourse.bass as bass
import concourse.bacc as bacc
import concourse.tile as tile
import concourse.mybir as mybir
from concourse.bass_utils import run_bass_kernel_spmd

BF16 = ml_dtypes.bfloat16
AF = mybir.ActivationFunctionType
OP = mybir.AluOpType

N, E, DIN, DOUT, DE = 100000, 1000000, 128, 64, 11
NEG_SLOPE = 0.2
NC = 8
NOWN = N // NC                    # 12500
NCHUNK = 98                       # ceil(12500/128)
NPAD = NCHUNK * 128               # 12544
BANKS = 4
BANKROWS = 25024                  # 4*25024 = 100096 >= N; < 2^15 for int16 idx
XT_COLS = 100352                  # 98*1024 table-build granularity (>= 100096)
RHS_W = 2 + DE + DOUT             # [1 | ex | attr(11) | ex*xl(64)] = 77

last_exec_time_ns = None
last_insts = None
_CACHE = {}


def _cdiv(a, b):
    return -(-a // b)


def _bc3(ap2, mid):
    """[P, F] AP -> [P, mid, F] AP broadcast along a new middle dim."""
    return bass.AP(ap2.tensor, ap2.offset, [ap2.ap[0], [0, mid], ap2.ap[1]])


def _in3(ap2, inner):
    """[P, T] AP -> [P, T, inner] AP broadcast along a new inner dim."""
    return bass.AP(ap2.tensor, ap2.offset, [ap2.ap[0], ap2.ap[1], [0, inner]])


# --------------------------------------------------------------------------
# host-side layout (index manipulation only)
# --------------------------------------------------------------------------

def _plan(edge_index):
    src = np.asarray(edge_index[0]).astype(np.int64)
    dst = np.asarray(edge_index[1]).astype(np.int64)
    core = dst // NOWN
    ldst = dst - core * NOWN
    chunk = ldst >> 7
    bank = src // BANKROWS
    cell = (core * NCHUNK + chunk) * BANKS + bank
    order = np.argsort(cell, kind="stable")
    counts = np.bincount(cell, minlength=NC * NCHUNK * BANKS)
    maxcnt = counts.reshape(NC, NCHUNK, BANKS).max(axis=0)   # [98,4] shared
    ntiles_cb = (maxcnt + 127) // 128
    t_ch = ntiles_cb.sum(axis=1)
    tile_base = np.zeros(NCHUNK + 1, np.int64)
    tile_base[1:] = np.cumsum(t_ch)
    cell_tile_ofs = np.cumsum(ntiles_cb, axis=1) - ntiles_cb
    starts = np.zeros(NC * NCHUNK * BANKS + 1, np.int64)
    starts[1:] = np.cumsum(counts)
    cell_s = cell[order]
    rank = np.arange(E, dtype=np.int64) - starts[cell_s]
    return dict(src=src, dst=dst, ldst=ldst, order=order, cell_s=cell_s,
                rank=rank, maxcnt=maxcnt, ntiles_cb=ntiles_cb, t_ch=t_ch,
                tile_base=tile_base, cell_tile_ofs=cell_tile_ofs,
                tot_tiles=int(t_ch.sum()))


def _host_arrays(plan, edge_attr):
    tot_tiles = plan["tot_tiles"]
    idxcols = tot_tiles * 8
    order, cell_s, rank = plan["order"], plan["cell_s"], plan["rank"]
    tile_base, cto = plan["tile_base"], plan["cell_tile_ofs"]

    core_s = cell_s // (NCHUNK * BANKS)
    ch_s = (cell_s // BANKS) % NCHUNK
    b_s = cell_s % BANKS
    cell_t0 = tile_base[ch_s] + cto[ch_s, b_s]
    tile_abs = cell_t0 + rank // 128
    part = rank % 128
    ofs16 = cell_t0 * 8 + rank // 16

    src_o = plan["src"][order]
    ldst_o = plan["ldst"][order]
    attr_o = np.asarray(edge_attr)[order]

    per_core = []
    for c in range(NC):
        m = core_s == c
        t_, p_ = tile_abs[m], part[m]
        r16 = rank[m] % 16
        o16 = ofs16[m]

        i16 = np.zeros((16, idxcols), np.int16)
        i16[r16, o16] = (src_o[m] - b_s[m] * BANKROWS).astype(np.int16)
        idx_xl = np.tile(i16, (8, 1))

        ldst_rel = np.full((128, tot_tiles), -1.0, BF16)
        ldst_rel[p_, t_] = (ldst_o[m] & 127).astype(BF16)

        attr_row = np.zeros((128, tot_tiles, DE), np.float32)
        attr_row[p_, t_, :] = attr_o[m]

        attr_t = np.zeros((DE, tot_tiles * 128), np.float32)
        attr_t[:, t_ * 128 + p_] = attr_o[m].T

        per_core.append(dict(idx_xl=idx_xl, ldst_rel=ldst_rel,
                             attr_row=attr_row.reshape(128, tot_tiles * DE),
                             attr_t=attr_t))
    return per_core


# --------------------------------------------------------------------------
# device program (one SPMD NEFF for 8 cores; layout baked from `plan`)
# --------------------------------------------------------------------------

def _build_device(plan):
    dt = mybir.dt
    maxcnt = plan["maxcnt"]
    ntiles_cb = plan["ntiles_cb"]
    t_ch = plan["t_ch"]
    tile_base = plan["tile_base"]
    cto = plan["cell_tile_ofs"]
    tot_tiles = plan["tot_tiles"]
    tmax = int(t_ch.max())
    assert tmax <= 16, f"psM bank budget: tmax={tmax}"

    nc = bacc.Bacc("TRN2", target_bir_lowering=False, debug=False,
                   num_devices=NC, num_swdge_queues=4)

    xT = nc.dram_tensor("xT", [128, XT_COLS], dt.float32, kind="ExternalInput")
    x_ownT = nc.dram_tensor("x_ownT", [128, NPAD], dt.float32, kind="ExternalInput")
    w_cat = nc.dram_tensor("w_cat", [128, 128], dt.float32, kind="ExternalInput")
    w_e = nc.dram_tensor("w_e", [DE, DOUT], dt.float32, kind="ExternalInput")
    att_bc = nc.dram_tensor("att_bc", [128, DOUT], dt.float32, kind="ExternalInput")
    idx_xl = nc.dram_tensor("idx_xl", [128, tot_tiles * 8], dt.int16, kind="ExternalInput")
    ldst_rel = nc.dram_tensor("ldst_rel", [128, tot_tiles], dt.bfloat16, kind="ExternalInput")
    attr_row = nc.dram_tensor("attr_row", [128, tot_tiles * DE], dt.float32, kind="ExternalInput")
    attr_t = nc.dram_tensor("attr_t", [DE, tot_tiles * 128], dt.float32, kind="ExternalInput")
    out_d = nc.dram_tensor("out", [NOWN, DOUT], dt.float32, kind="ExternalOutput")

    qn = [0]

    def next_q():
        q = qn[0] & 3
        qn[0] += 1
        return q

    with tile.TileContext(nc) as tc:
        with (
            tc.tile_pool(name="const", bufs=1) as constp,
            tc.tile_pool(name="own", bufs=1) as ownp,
            tc.tile_pool(name="dram", bufs=1, space="DRAM") as dramp,
            tc.tile_pool(name="tload", bufs=3) as tloadp,
            tc.tile_pool(name="stream", bufs=2) as streamp,
            tc.tile_pool(name="gat", bufs=2) as gatp,
            tc.tile_pool(name="work", bufs=2) as workp,
            tc.tile_pool(name="spool", bufs=3) as spool,
            tc.tile_pool(name="epi", bufs=1) as epip,
            tc.tile_pool(name="psA", bufs=2, space="PSUM") as psA,
            tc.tile_pool(name="psT", bufs=2, space="PSUM") as psT,
            tc.tile_pool(name="psM", bufs=2, space="PSUM") as psM,
        ):
            # ---------------- constants
            iota_i = constp.tile([128, 128], dt.int16, tag="iota_i")
            nc.gpsimd.iota(iota_i[:], pattern=[[1, 128]], base=0, channel_multiplier=0)
            iota_b = constp.tile([128, 128], dt.bfloat16, tag="iota_b")
            nc.vector.tensor_copy(iota_b[:], iota_i[:])

            iota_ri = constp.tile([128, tmax, 128], dt.int16, tag="iota_ri")
            nc.gpsimd.iota(iota_ri[:, :, :], pattern=[[0, tmax], [1, 128]], base=0,
                           channel_multiplier=0)
            iota_rep = constp.tile([128, tmax, 128], dt.bfloat16, tag="iota_rep")
            nc.vector.tensor_copy(iota_rep[:, :, :], iota_ri[:, :, :])

            iota_pm = constp.tile([128, 128], dt.int16, tag="iota_pm")
            nc.gpsimd.iota(iota_pm[:], pattern=[[1, 128]], base=0, channel_multiplier=-1)
            iota_pmf = constp.tile([128, 128], dt.float32, tag="iota_pmf")
            nc.vector.tensor_copy(iota_pmf[:], iota_pm[:])
            ident_b = constp.tile([128, 128], dt.bfloat16, tag="ident_b")
            nc.vector.tensor_scalar(out=ident_b[:], in0=iota_pmf[:], scalar1=0.0,
                                    scalar2=None, op0=OP.is_equal)
            ident_f = constp.tile([128, 128], dt.float32, tag="ident_f")
            nc.vector.tensor_scalar(out=ident_f[:], in0=iota_pmf[:], scalar1=0.0,
                                    scalar2=None, op0=OP.is_equal)

            wcat_f = constp.tile([128, 128], dt.float32, tag="wcat_f")
            nc.sync.dma_start(wcat_f[:], w_cat[:])
            wcat_b = constp.tile([128, 128], dt.bfloat16, tag="wcat_b")
            nc.vector.tensor_copy(wcat_b[:], wcat_f[:])
            we_f = constp.tile([DE, DOUT], dt.float32, tag="we_f")
            nc.sync.dma_start(we_f[:], w_e[:])
            attb_f = constp.tile([128, DOUT], dt.float32, tag="attb_f")
            nc.sync.dma_start(attb_f[:], att_bc[:])
            attb_b = constp.tile([128, DOUT], dt.bfloat16, tag="attb_b")
            nc.vector.tensor_copy(attb_b[:], attb_f[:])

            xl_own = ownp.tile([128, NCHUNK, DOUT], dt.bfloat16, tag="xl_own")
            xr_own = ownp.tile([128, NCHUNK, DOUT], dt.bfloat16, tag="xr_own")
            agg_all = ownp.tile([128, NCHUNK, RHS_W], dt.float32, tag="agg_all")
            ldst_sb = ownp.tile([128, tot_tiles], dt.bfloat16, tag="ldst_sb")
            nc.sync.dma_start(ldst_sb[:], ldst_rel[:])

            C_tab = dramp.tile([XT_COLS, 128], dt.bfloat16)

            # ---------------- phase 1a: full table C[n] = [xl(n) | xr(n)]
            for i in range(XT_COLS // 1024):
                xt_f = tloadp.tile([128, 1024], dt.float32, tag="xt_f")
                nc.sync.dma_start(xt_f[:], xT[:, i * 1024:(i + 1) * 1024])
                xt_b = tloadp.tile([128, 1024], dt.bfloat16, tag="xt_b")
                nc.vector.tensor_copy(xt_b[:], xt_f[:])
                for j in range(2):
                    ps = psM.tile([128, 512], dt.float32, tag="psM")
                    for k in range(4):
                        nc.tensor.matmul(
                            ps[:, k * 128:(k + 1) * 128],
                            lhsT=xt_b[:, (j * 4 + k) * 128:(j * 4 + k + 1) * 128],
                            rhs=wcat_b[:], start=True, stop=True)
                    cs = tloadp.tile([128, 512], dt.bfloat16, tag="cs")
                    nc.scalar.copy(cs[:], ps[:])
                    for k in range(4):
                        n0 = (i * 8 + j * 4 + k) * 128
                        nc.sync.dma_start(C_tab[n0:n0 + 128, :],
                                          cs[:, k * 128:(k + 1) * 128])

            # ---------------- phase 1b: own-node tiles + XR table [xr | xl]
            for ch in range(NCHUNK):
                xo_f = tloadp.tile([128, 128], dt.float32, tag="xo_f")
                nc.sync.dma_start(xo_f[:], x_ownT[:, ch * 128:(ch + 1) * 128])
                xo_b = tloadp.tile([128, 128], dt.bfloat16, tag="xo_b")
                nc.vector.tensor_copy(xo_b[:], xo_f[:])
                pso = psA.tile([128, 128], dt.float32, tag="psA")
                nc.tensor.matmul(pso[:], lhsT=xo_b[:], rhs=wcat_b[:],
                                 start=True, stop=True)
                cso = tloadp.tile([128, 128], dt.bfloat16, tag="cso")
                nc.scalar.copy(cso[:], pso[:])
                nc.vector.tensor_copy(xl_own[:, ch, :], cso[:, 0:DOUT])
                nc.vector.tensor_copy(xr_own[:, ch, :], cso[:, DOUT:128])

            # ---------------- phase 2: per-chunk edge pipeline
            for ch in range(NCHUNK):
                T = int(t_ch[ch])
                if T == 0:
                    nc.vector.memset(agg_all[:, ch, :], 0.0)
                    continue
                tb = int(tile_base[ch])

                idxl = streamp.tile([128, tmax * 8], dt.int16, tag="idxl")
                nc.sync.dma_start(idxl[:, 0:T * 8], idx_xl[:, tb * 8:(tb + T) * 8])
                atr = streamp.tile([128, tmax * DE], dt.float32, tag="atr")
                nc.sync.dma_start(atr[:, 0:T * DE], attr_row[:, tb * DE:(tb + T) * DE])
                att_t = streamp.tile([DE, tmax * 128], dt.float32, tag="att_t")
                nc.sync.dma_start(att_t[:, 0:T * 128], attr_t[:, tb * 128:(tb + T) * 128])

                g = gatp.tile([128, tmax, 128], dt.bfloat16, tag="g")
                if ch < 2:  # first uses of the rotating slots: clear stale bits
                    nc.vector.memset(g[:, :, :], 0.0)
                for b in range(BANKS):
                    ncb = int(ntiles_cb[ch, b])
                    if ncb == 0:
                        continue
                    mc = int(maxcnt[ch, b])
                    bofs = int(cto[ch, b])
                    nc.gpsimd.dma_gather(
                        out_ap=g[:, bofs:bofs + ncb, :],
                        in_ap=C_tab[b * BANKROWS:(b + 1) * BANKROWS, :],
                        idxs_ap=idxl[:, (bofs) * 8:(bofs) * 8 + _cdiv(mc, 16)],
                        num_idxs=mc, num_idxs_reg=mc, elem_size=128,
                        queue_num=next_q())

                # one-hot S for every tile of the chunk, one DVE op
                s_all = workp.tile([128, tmax, 128], dt.bfloat16, tag="s_all")
                nc.vector.tensor_tensor(out=s_all[:, 0:T, :],
                                        in0=iota_rep[:, 0:T, :],
                                        in1=_in3(ldst_sb[:, tb:tb + T], 128),
                                        op=OP.is_equal)

                # message assembly: psum m' = xr[dst] + attr @ We ; m = xl[src] + m'
                psm = psM.tile([128, tmax * 64], dt.float32, tag="psM")
                for t in range(T):
                    sl = psm[:, t * 64:(t + 1) * 64]
                    sje_p = psT.tile([128, 128], dt.bfloat16, tag="psT")
                    nc.tensor.transpose(sje_p[:], in_=s_all[:, t, :],
                                        identity=ident_b[:])
                    sje = spool.tile([128, 128], dt.bfloat16, tag="sje")
                    nc.scalar.copy(sje[:], sje_p[:])
                    nc.tensor.matmul(sl, lhsT=sje[:], rhs=xr_own[:, ch, :],
                                     start=True, stop=False)
                    nc.tensor.matmul(sl, lhsT=att_t[:, t * 128:(t + 1) * 128],
                                     rhs=we_f[:], start=False, stop=True)

                mrelu = workp.tile([128, tmax * 64], dt.bfloat16, tag="mrelu")
                mr3v = psm[:, 0:T * 64].rearrange("p (t d) -> p t d", d=64)
                nc.vector.tensor_tensor(out=mrelu[:, 0:T * 64].rearrange(
                    "p (t d) -> p t d", d=64), in0=mr3v, in1=g[:, 0:T, 0:DOUT],
                    op=OP.add)
                nc.scalar.activation(mrelu[:, 0:T * 64], mrelu[:, 0:T * 64],
                                     AF.Prelu, alpha=NEG_SLOPE)
                lt = workp.tile([128, tmax, 64], dt.bfloat16, tag="lt")
                mr3 = mrelu[:, 0:T * 64].rearrange("p (t d) -> p t d", d=64)
                nc.vector.tensor_tensor(out=lt[:, 0:T, :], in0=mr3,
                                        in1=_bc3(attb_b[:, :], T), op=OP.mult)
                logits = workp.tile([128, tmax], dt.float32, tag="logits")
                nc.vector.tensor_reduce(out=logits[:, 0:T], in_=lt[:, 0:T, :],
                                        axis=mybir.AxisListType.X, op=OP.add)
                ex = workp.tile([128, tmax], dt.bfloat16, tag="ex")
                nc.scalar.activation(ex[:, 0:T], logits[:, 0:T], AF.Exp)

                # scatter rhs = [1 | ex | attr | ex*xl]
                rhs = workp.tile([128, tmax, RHS_W], dt.bfloat16, tag="rhs")
                nc.vector.memset(rhs[:, 0:T, 0:1], 1.0)
                nc.vector.tensor_copy(rhs[:, 0:T, 1:2], _in3(ex[:, 0:T], 1))
                atr3 = atr[:, 0:T * DE].rearrange("p (t a) -> p t a", a=DE)
                nc.vector.tensor_copy(rhs[:, 0:T, 2:2 + DE], atr3)
                g3 = g[:, 0:T, 0:DOUT]
                nc.vector.tensor_tensor(out=rhs[:, 0:T, 2 + DE:RHS_W], in0=g3,
                                        in1=_in3(ex[:, 0:T], DOUT), op=OP.mult)

                agg = psA.tile([128, RHS_W], dt.float32, tag="psA")
                for t in range(T):
                    nc.tensor.matmul(agg[:], lhsT=s_all[:, t, :], rhs=rhs[:, t, :],
                                     start=(t == 0), stop=(t == T - 1))
                nc.vector.tensor_copy(agg_all[:, ch, :], agg[:])

            # ---------------- epilogue (self-loop terms; batched over chunks)
            deg = epip.tile([128, NCHUNK], dt.float32, tag="deg")
            nc.vector.tensor_scalar(out=deg[:], in0=agg_all[:, :, 0], scalar1=1.0,
                                    scalar2=None, op0=OP.max)
            rdeg = epip.tile([128, NCHUNK], dt.float32, tag="rdeg")
            nc.vector.reciprocal(rdeg[:], deg[:])
            # lep[ch] = (sum_seg attr) @ W_e, via per-chunk PE transpose of the
            # [128, 11] attr-sum slab
            lep = epip.tile([128, NCHUNK, DOUT], dt.float32, tag="lep")
            for ch in range(NCHUNK):
                pst = psT.tile([DE, 128], dt.float32, tag="psT")
                nc.tensor.transpose(pst[:], in_=agg_all[:, ch, 2:2 + DE],
                                    identity=ident_f[:])
                laT = spool.tile([DE, 128], dt.float32, tag="laT")
                nc.vector.tensor_copy(laT[:], pst[:])
                psl = psT.tile([128, DOUT], dt.float32, tag="psT")
                nc.tensor.matmul(psl[:], lhsT=laT[:], rhs=we_f[:],
                                 start=True, stop=True)
                nc.vector.tensor_copy(lep[:, ch, :], psl[:])

            # m_loop = xl_own + xr_own + lep/deg   (reuse lep in place)
            nc.vector.tensor_tensor(out=lep[:], in0=lep[:],
                                    in1=_in3(rdeg[:, :], DOUT), op=OP.mult)
            nc.vector.tensor_tensor(out=lep[:], in0=lep[:], in1=xl_own[:],
                                    op=OP.add)
            nc.vector.tensor_tensor(out=lep[:], in0=lep[:], in1=xr_own[:],
                                    op=OP.add)
            mlr = epip.tile([128, NCHUNK, DOUT], dt.bfloat16, tag="mlr")
            nc.scalar.activation(mlr[:], lep[:], AF.Prelu, alpha=NEG_SLOPE)
            nc.vector.tensor_tensor(out=mlr[:], in0=mlr[:],
                                    in1=_bc3(attb_b[:, :], NCHUNK), op=OP.mult)
            exl = epip.tile([128, NCHUNK], dt.float32, tag="exl")
            nc.vector.tensor_reduce(out=exl[:], in_=mlr[:],
                                    axis=mybir.AxisListType.X, op=OP.add)
            nc.scalar.activation(exl[:], exl[:], AF.Exp)
            rden = epip.tile([128, NCHUNK], dt.float32, tag="rden")
            nc.vector.tensor_tensor(out=rden[:], in0=agg_all[:, :, 1], in1=exl[:],
                                    op=OP.add)
            nc.vector.reciprocal(rden[:], rden[:])
            o1 = epip.tile([128, NCHUNK, DOUT], dt.float32, tag="lep")  # reuse slot
            nc.vector.tensor_tensor(out=o1[:], in0=xl_own[:],
                                    in1=_in3(exl[:, :], DOUT), op=OP.mult)
            nc.vector.tensor_tensor(out=o1[:], in0=o1[:],
                                    in1=agg_all[:, :, 2 + DE:RHS_W], op=OP.add)
            nc.vector.tensor_tensor(out=o1[:], in0=o1[:],
                                    in1=_in3(rden[:, :], DOUT), op=OP.mult)
            for ch in range(NCHUNK):
                rows = min(128, NOWN - ch * 128)
                nc.sync.dma_start(out_d[ch * 128:ch * 128 + rows, :],
                                  o1[0:rows, ch, :])

    nc.compile()
    return nc


# --------------------------------------------------------------------------
# entry point
# --------------------------------------------------------------------------

def kernel(x, edge_index, edge_attr, W_l, W_r, W_e, att):
    global last_exec_time_ns
    x = np.asarray(x, np.float32)
    edge_attr = np.asarray(edge_attr, np.float32)
    W_l = np.asarray(W_l, np.float32)
    W_r = np.asarray(W_r, np.float32)
    W_e = np.asarray(W_e, np.float32)
    att = np.asarray(att, np.float32)

    plan = _plan(edge_index)
    per_core = _host_arrays(plan, edge_attr)

    key = plan["maxcnt"].tobytes()
    if key not in _CACHE:
        _CACHE[key] = _build_device(plan)
    nc = _CACHE[key]

    xT = np.zeros((128, XT_COLS), np.float32)
    xT[:, :N] = x.T
    w_cat = np.concatenate([W_l, W_r], axis=1)
    att_bc = np.tile(att[None, :], (128, 1)).astype(np.float32)

    in_maps = []
    for c in range(NC):
        x_ownT = np.zeros((128, NPAD), np.float32)
        x_ownT[:, :NOWN] = x[c * NOWN:(c + 1) * NOWN].T
        pc = per_core[c]
        in_maps.append({
            "xT": xT, "x_ownT": x_ownT, "w_cat": w_cat, "w_e": W_e,
            "att_bc": att_bc, "idx_xl": pc["idx_xl"],
            "ldst_rel": pc["ldst_rel"], "attr_row": pc["attr_row"],
            "attr_t": pc["attr_t"],
        })

    global last_insts
    try:
        res = run_bass_kernel_spmd(nc, in_maps, core_ids=list(range(NC)), trace=True)
        last_exec_time_ns = res.exec_time_ns
        last_insts = res.instructions_and_trace[0] if res.instructions_and_trace else None
    except Exception:
        res = run_bass_kernel_spmd(nc, in_maps, core_ids=list(range(NC)), trace=False)
        last_exec_time_ns = None
        last_insts = None

    return np.concatenate([res.results[c]["out"] for c in range(NC)], axis=0)

